# revision 37
# baseline (speedup 1.0000x reference)
"""Trainium2 Bass kernel: GPT-2 style causal attention + output projection.

Reference computation (B=2, L=2048, D=1024, H=16, dh=64):
    q,k,v = split_heads(query/key/value)            # [B,H,L,dh]
    S = q @ k^T / sqrt(dh)                          # [B,H,L,L]
    P = softmax(causal_mask(S))
    A = merge_heads(P @ v)                          # [B,L,D]
    out = A @ w_proj + b_proj
Sharding: 32 (b,h) pairs, 4 per core (cores 0-3 batch 0, 4-7 batch 1).
Each core computes attention for its 4 heads and a partial c_proj using its
256 rows of w_proj; the host sums the 4 partials per batch.

Scores are computed transposed (S^T, keys on partitions) so softmax's P
lands with keys on the partition axis -- the layout P.V needs.  A ones
column in V makes the PV matmul also emit softmax denominators.

v2 changes over the 125us baseline (ACT was 87% busy, PE 73%):
 - exp is SPLIT between the Scalar engine (activation Exp) and a custom
   Vector-engine op EXP2A_ANT: a Schraudolph-style exp2 that computes the
   fp16 BIT PATTERN of 2^z as a fixed-point fp32 value and lets the
   int16 output-conversion do the float->int step (a |frac| quadratic
   fits the round-to-nearest split's kink; 0.31% max rel err -- the DVE
   ALU has no float->int op and ABSOLUTE_DIFF gets |frac| in one stage).
   Scores arrive pre-scaled by 2^10*log2(e)/8 via a host-side qt scale,
   so both engines read the same PSUM scores: ACT exps with
   scale=ln2/2^10, DVE works on the fixed-point directly.  Long q-blocks
   keep diagonal (trimmed+masked) chunks on ACT; short blocks and
   sub-diagonal chunks alternate ACT/DVE 3:2.  Hard-won firmware notes:
   streaming-Src1 custom-DVE ops and base_partition!=0 custom-DVE ops
   both silently misbehave, so the op uses the C3-latch ([P,1] in1) for
   its 4th constant and all custom-DVE inputs sit at partition 0.
 - V's zero-padding columns 0..63 are ALL ONES: the PV matmul broadcasts
   the softmax denominator into PSUM partitions 0..63 for free (those
   array columns were multiplying zeros), dh lands at 64..127.  The
   normalize chain collapses to fast-reciprocal [64,512] + tensor_mul,
   both DVE, no PE broadcast matmul and no cross-engine hops.
 - QK chunk pairs batched x2: the 64-row QK phases and 128-row PV/c_proj
   phases alternate half as often, halving PE array fill/drain switches.
 - c_proj emits one 128-query row-block per group (4 matmuls sharing
   each atn stationary, one 2-bank PSUM tile, one cast, one [128,1024]
   DMA); casts alternate DVE/ACT, out-DMA triggers alternate sync/gpsimd.
 - triangular band masks run on gpsimd (SBUF-only; gpsimd cannot touch
   PSUM), which is otherwise idle.
 - the final flush interleaves dependency-free warm matmuls so the PE's
   HAM clock gate stays at 2.4GHz through the last normalize chains.

Scheduling: QK runs two chunks ahead of PV (3 score buffers), each duo's
tail PVs and normalize chains are deferred into the next duo's chunk
stream, and each q-block's c_proj is spread one group at a time through
the next block.  Deferred work MUST be emitted in dependency order.
q-blocks run largest-first (J=3..0).  Diagonal trimming: key-chunks with
m = c-4J >= 1 skip the dead q < m*128 region; the multiplicative mask is
a single 128x128 triangle applied to the mixed diagonal band only.

Precision: fp16 matmul operands, fp32 PSUM accumulation; exp 0.31% (DVE)
/ exact (ACT); fast-reciprocal ~51 ULP fp16; fp16 out partials.  Total
~6e-4 max-abs relative error against a 2e-2 budget.
"""

import numpy as np

B, L, D, H = 2, 2048, 1024, 16
DH = 64          # head dim
PAIRS = 4        # (b,h) pairs per core
QB = 512         # query block
KC = 128         # key chunk
NCORES = 8

# --- custom DVE exp2 constants (see work/derisk notes) ---
MAGIC = 1.5 * 2 ** 33          # round-to-nearest at 2^10 granularity
B2, B1, B0 = 0.355, -0.35125, 0.00175   # p(|f|) = B2 f^2 + B1|f| (+B0)
SCALE_Q = 1024.0 * 0.125 * 1.4426950408889634   # 2^10 * log2(e)/sqrt(dh)
LN2_1024 = 0.6931471805599453 / 1024.0
# sub-diagonal chunks cycle through this pattern: True -> DVE exp
DVE_SHARE = (False, True, False, False, True)

_COMPILED = None
_EXP2A = None


def _register_exp2():
    global _EXP2A
    if _EXP2A is not None:
        return _EXP2A
    import concourse.dve_ops as dops
    for op in dops.OPS:
        if op.name == "EXP2A_ANT":
            _EXP2A = op
            return op
    from concourse.dve_spec import (
        Spec, Src0, C0, C1, C2, C3, lower, _spill_c3_to_src1, Bin, AluOp,
    )
    from concourse.dve_spec import _has_src1 as has_src1
    from concourse.dve_uop import DveOpSpec

    t = Src0 + C0
    nf = t - C0
    u = Bin(AluOp.ABSOLUTE_DIFF, Src0, nf)
    q3 = (u * C1 + C2) * u
    w = Src0 + C3
    body = _spill_c3_to_src1(w + q3)

    def ref(in0, in1, s0, s1, imm2):
        tt = (in0.astype(np.float32) + s0).astype(np.float32)
        nfv = (tt - s0).astype(np.float32)
        uv = np.abs(in0 - nfv)
        q3v = ((uv * s1 + imm2) * uv).astype(np.float32)
        return (in0 + in1[:, :1]) + q3v

    spec = Spec(body=body, reference=ref)
    rd1 = has_src1(spec)
    shas = {}
    for ver in ("v3", "v4"):
        shas[ver] = DveOpSpec(
            name="EXP2A_ANT", opcode=0, uops=lower(spec, ver=ver), rd1_en=rd1
        ).sha(ver)
    op = dops.DveOp("EXP2A_ANT", spec, subdim=False, uops_sha=shas)
    dops.OPS.append(op)
    dops._SUB_OPCODE_FOR_NAME["EXP2A_ANT"] = (
        dops._CUSTOM_DVE_ROW_BASE + len(dops.OPS) - 1)
    assert dops._SUB_OPCODE_FOR_NAME["EXP2A_ANT"] < 0x20
    dops.CUSTOM_DVE_SPECS["EXP2A_ANT"] = spec
    _EXP2A = op
    return op


def _build_nc():
    import concourse.bacc as bacc
    import concourse.tile as tile
    from concourse import mybir

    exp2a = _register_exp2()

    f32 = mybir.dt.float32
    f16 = mybir.dt.float16
    i16 = mybir.dt.int16
    Exp = mybir.ActivationFunctionType.Exp

    nc = bacc.Bacc("TRN2", target_bir_lowering=False, debug=False,
                   num_devices=NCORES)

    qt_d = nc.dram_tensor("qt", [2, 128, L], f16, kind="ExternalInput").ap()
    kt_d = nc.dram_tensor("kt", [2, 128, L], f16, kind="ExternalInput").ap()
    v_d = nc.dram_tensor("v", [PAIRS, 128, (L // KC) * 128], f16,
                         kind="ExternalInput").ap()
    masks_d = nc.dram_tensor("masks", [128, 2 * KC], f16, kind="ExternalInput").ap()
    w_d = nc.dram_tensor("w", [2, 128, D], f16, kind="ExternalInput").ap()
    out_d = nc.dram_tensor("out", [L, D], f16, kind="ExternalOutput").ap()

    with tile.TileContext(nc) as tc:
        with (
            tc.tile_pool(name="consts", bufs=1) as consts,
            tc.tile_pool(name="st", bufs=3, space="PSUM") as st_pool,
            tc.tile_pool(name="at", bufs=2, space="PSUM") as at_pool,
            tc.tile_pool(name="et", bufs=12) as et_pool,
            tc.tile_pool(name="atn", bufs=8) as atn_pool,
            tc.tile_pool(name="rbc", bufs=8) as rbc_pool,
            tc.tile_pool(name="osb", bufs=8) as osb_pool,
        ):
            # resident inputs
            qt = [consts.tile([128, L], f16, name=f"qt{i}", tag=f"qt{i}")
                  for i in range(2)]
            kt = [consts.tile([128, L], f16, name=f"kt{i}", tag=f"kt{i}")
                  for i in range(2)]
            vt = [consts.tile([128, (L // KC) * 128], f16,
                              name=f"vt{i}", tag=f"vt{i}") for i in range(PAIRS)]
            mk = consts.tile([128, 2 * KC], f16, name="mk", tag="mk")
            wt = [consts.tile([128, D], f16, name=f"wt{i}", tag=f"wt{i}")
                  for i in range(2)]
            k1 = consts.tile([128, 1], f32, name="k1", tag="k1")

            # J=3/duo0 runs first, so its kt slice and qt tail land first.
            # The first-needed slices fan out across five DMA queues so
            # their dispatch+transfer run in parallel, not serialized on
            # the sync ring.
            nc.sync.dma_start(kt[0][:, 0:128], kt_d[0][:, 0:128])
            nc.scalar.dma_start(qt[0][0:64, 1536:2048], qt_d[0][0:64, 1536:2048])
            nc.gpsimd.dma_start(qt[0][64:128, 1536:2048],
                                qt_d[0][64:128, 1536:2048])
            nc.scalar.dma_start(kt[0][:, 128:512], kt_d[0][:, 128:512])
            nc.gpsimd.dma_start(vt[0][:], v_d[0])
            nc.gpsimd.dma_start(vt[1][:], v_d[1])
            nc.sync.dma_start(kt[0][:, 512:768], kt_d[0][:, 512:768])
            nc.sync.dma_start(kt[0][:, 768:1024], kt_d[0][:, 768:1024])
            nc.sync.dma_start(kt[0][:, 1024:2048], kt_d[0][:, 1024:2048])
            nc.sync.dma_start(mk[:], masks_d[:])
            nc.sync.dma_start(qt[1][:], qt_d[1])
            nc.sync.dma_start(kt[1][:], kt_d[1])
            for p in range(2, PAIRS):
                nc.sync.dma_start(vt[p][:], v_d[p])
            nc.sync.dma_start(qt[0][:, 0:1536], qt_d[0][:, 0:1536])
            for i in range(2):
                nc.sync.dma_start(wt[i][:], w_d[i])



            # HAM pre-warm: keep the PE activity monitor busy through the
            # initial DMA wait (64-row mode so the first real QK needs no
            # mode-switch drain); PSUM scratch recycled by the st pool.
            warm = consts.tile([64, 640], f16, name="warm", tag="warm")
            nc.vector.memset(warm[:], 0.0)
            nc.vector.memset(k1[:], (15.0 + B0) * 1024.0)
            wps = st_pool.tile([128, 512], f32, name="wps", tag="st")
            for _ in range(6):
                nc.tensor.matmul(
                    wps[:], lhsT=warm[:, 512:640], rhs=warm[:, 0:512],
                    start=True, stop=True, tile_position=(0, 0),
                )

            def cproj_groups(J, atn_duo, final=False):
                # one group = a full 128-query output row-block: 4 matmuls
                # sharing each atn stationary (2 LDWs instead of 4), one
                # 2-bank cp tile, one cast, one DMA
                def one(rt):
                    def emit():
                        cp = st_pool.tile([128, 2 * 512], f32, name="cp",
                                          tag="st")
                        for duo in range(2):
                            for nf in range(2):
                                nc.tensor.matmul(
                                    cp[:, nf * 512:(nf + 1) * 512],
                                    lhsT=atn_duo[duo][:, rt * 128:(rt + 1) * 128],
                                    rhs=wt[duo][:, nf * 512:(nf + 1) * 512],
                                    start=(duo == 0), stop=(duo == 1),
                                )
                        ob = osb_pool.tile([128, 1024], f16, name="ob",
                                           tag="ob")
                        if final:
                            # flush: split the cast across both engines so
                            # the tail drains in parallel
                            nc.vector.tensor_copy(ob[:, 0:512], cp[:, 0:512])
                            nc.scalar.copy(ob[:, 512:1024], cp[:, 512:1024])
                        elif rt % 2:
                            nc.scalar.copy(ob[:], cp[:])
                        else:
                            nc.vector.tensor_copy(ob[:], cp[:])
                        dq = nc.gpsimd if rt % 2 else nc.sync
                        dq.dma_start(
                            out_d[J * QB + rt * 128:J * QB + (rt + 1) * 128, :],
                            ob[:],
                        )
                    return emit
                return [one(rt) for rt in range(QB // 128)]

            pending = []             # c_proj groups from the previous q-block
            pending_need = 0         # norm-chain count its groups depend on
            pending_norm = []        # softmax-normalize chains, deferred
            pending_pv = []          # tail PVs of the previous duo
            norms_made = 0
            norms_run = [0]
            nsub = [0]               # sub-diagonal chunk counter (exp split)
            for J in reversed(range(L // QB)):
                nch = 4 * J + 4      # causal: key chunks 0..nch-1
                atn_duo = []
                for duo in range(2):
                    at = [at_pool.tile([128, QB], f32, name="at", tag="at")
                          for _ in range(2)]
                    ets = {}

                    def emit_qk(c):
                        st = st_pool.tile([128, 2 * QB], f32, name="st",
                                          tag="st")
                        m = c - 4 * J
                        lo = m * KC if m >= 1 else 0
                        for h2 in range(2):
                            nc.tensor.matmul(
                                st[:, h2 * QB + lo:(h2 + 1) * QB],
                                lhsT=kt[duo][64 * h2:64 * (h2 + 1),
                                             c * KC:(c + 1) * KC],
                                rhs=qt[duo][64 * h2:64 * (h2 + 1),
                                            J * QB + lo:(J + 1) * QB],
                                start=True, stop=True,
                                tile_position=(64 * h2, 0),
                            )
                        et = et_pool.tile([128, 2 * QB], f16, name="et",
                                          tag="et")
                        # diagonal chunks (m>=1): only q >= m*128 is causally
                        # valid; QK/exp/PV all skip the dead region.  Both
                        # engines handle the trimmed strided view.
                        if m >= 0 and J > 1:
                            use_dve = False   # long blocks: ACT absorbs diag
                        else:
                            use_dve = DVE_SHARE[nsub[0] % len(DVE_SHARE)]
                            nsub[0] += 1
                        if m >= 1:
                            ein = st[:].rearrange("p (h q) -> p h q", h=2)\
                                [:, :, m * KC:]
                            eout = et[:].rearrange("p (h q) -> p h q", h=2)\
                                [:, :, m * KC:]
                        else:
                            ein, eout = st[:], et[:]
                        if use_dve:
                            nc.vector._custom_dve(
                                exp2a, out=eout.bitcast(i16),
                                in0=ein, in1=k1[:],
                                s0=MAGIC, s1=B2 / 1024.0, imm2=B1)
                        else:
                            nc.scalar.activation(eout, ein, Exp,
                                                 scale=LN2_1024)
                        if m >= 0:
                            # 128-wide diagonal band is the only mixed
                            # valid/invalid region; one triangle serves all m
                            ev = et[:].rearrange("p (h q) -> p h q", h=2)\
                                [:, :, m * KC:(m + 1) * KC]
                            mv = mk[:].rearrange("p (h q) -> p h q", h=2)
                            nc.gpsimd.tensor_mul(ev, ev, mv)
                        ets[c] = et

                    def emit_pv(c, ets=ets, at=at, duo=duo, nch=nch, J=J):
                        et = ets.pop(c)
                        m = c - 4 * J
                        lo = m * KC if m >= 1 else 0
                        for h2 in range(2):
                            pair = 2 * duo + h2
                            nc.tensor.matmul(
                                at[h2][0:128, lo:QB],
                                lhsT=vt[pair][:, c * 128:(c + 1) * 128],
                                rhs=et[:, h2 * QB + lo:(h2 + 1) * QB],
                                start=(c == 0), stop=(c == nch - 1),
                            )

                    # chunks in batches of up to 3 (the st pool's depth):
                    # QK pairs back-to-back (one 64-row phase), then
                    # tail-PVs + the lagging PVs (watermark, uniform lag 3
                    # so no PV ever chases its own batch's exp) + popped
                    # c_proj groups (one contiguous 128-row phase) -- cuts
                    # the QK<->PV array fill/drain switches to 2 per batch
                    bsz, rem = [2], nch - 2
                    while rem >= 5:
                        bsz.append(3)
                        rem -= 3
                    if rem:
                        bsz.extend([rem] if rem <= 3 else [2, 2])
                    cc = 0
                    pvw = 0
                    for bs in bsz:
                        for c in range(cc, cc + bs):
                            emit_qk(c)
                        ncp = 0
                        for c in range(cc, cc + bs):
                            if pending_pv:
                                pending_pv.pop(0)()
                            elif pending_norm:
                                pending_norm.pop(0)()
                            elif (c >= 2 and pending
                                  and (nch < 12 or c % (nch // 4) == 0
                                       or c >= nch - 4)):
                                ncp += 1
                            if (nch == 4 and pending
                                    and norms_run[0] >= pending_need):
                                ncp += 1
                        while pvw < cc + bs - 4:
                            emit_pv(pvw)
                            pvw += 1
                        for _ in range(ncp):
                            if pending:
                                pending.pop(0)()
                        cc += bs
                    pending_pv.extend(
                        [lambda c=c, f=emit_pv: f(c)
                         for c in range(pvw, nch)])

                    atn = atn_pool.tile([128, QB], f16, name="atn", tag="atn")

                    def norm_one(h2, at=at, atn=atn):
                        # PSUM rows 0..63 hold 64 matmul-broadcast copies of
                        # the denominator; reciprocal + multiply, both DVE
                        def emit():
                            rbc = rbc_pool.tile([64, QB], f16, name="rbc",
                                                tag="rbc")
                            # fp16-out reciprocal_approx_fast (wrapper
                            # asserts fp32-out; the seed only needs the fp32
                            # INPUT bit layout, out-conversion is the write
                            # path's job)
                            from concourse.dve_ops import (
                                RECIPROCAL_APPROX_FAST, RECIP_APPROX_FAST_CONSTS)
                            nc.vector._custom_dve(
                                RECIPROCAL_APPROX_FAST, out=rbc[:],
                                in0=at[h2][0:64, :],
                                **RECIP_APPROX_FAST_CONSTS)
                            nc.vector.tensor_mul(
                                atn[64 * h2:64 * (h2 + 1), :],
                                at[h2][64:128, :], rbc[:])
                            norms_run[0] += 1
                        return emit

                    pending_norm.extend([norm_one(0), norm_one(1)])
                    norms_made += 2
                    atn_duo.append(atn)

                if J == 0:
                    leftovers = list(pending)
                    pending = cproj_groups(J, atn_duo, final=True)
                else:
                    for g in pending:   # stragglers from a short q-block
                        g()
                    pending = cproj_groups(J, atn_duo)
                pending_need = norms_made
            for g in pending_pv:
                g()
            for g in pending_norm:
                g()
            for g in leftovers:
                g()
            # HAM-warm filler: the final c_proj groups wait on the last
            # normalize chains; a few dependency-free matmuls keep the PE
            # activity monitor at full clock through that window
            for _ in range(6):
                wfil = st_pool.tile([128, 512], f32, name="wf", tag="st")
                nc.tensor.matmul(
                    wfil[:], lhsT=warm[:, 512:640], rhs=warm[:, 0:512],
                    start=True, stop=True, tile_position=(0, 0),
                )
            for g in pending:
                g()

    nc.compile()
    return nc


def _get_nc():
    global _COMPILED
    if _COMPILED is None:
        _COMPILED = _build_nc()
    return _COMPILED


def _prep_in_maps(query, key, value, w_proj):
    q = np.asarray(query, dtype=np.float32)
    k = np.asarray(key, dtype=np.float32)
    v = np.asarray(value, dtype=np.float32)
    w = np.asarray(w_proj, dtype=np.float32)

    q4 = q.reshape(B, L, H, DH) * SCALE_Q   # scores arrive as 2^10*log2e*s/8
    k4 = k.reshape(B, L, H, DH)
    v4 = v.reshape(B, L, H, DH)

    kp = np.arange(128)[:, None]
    jf = np.arange(KC)[None, :]
    tri = (kp <= jf).astype(np.float16)                      # [128, 128]
    masks = np.ascontiguousarray(np.concatenate([tri, tri], axis=1))

    in_maps = []
    for c in range(NCORES):
        b = c // 4
        hsel = 4 * (c % 4)
        qt = np.ascontiguousarray(
            q4[b].transpose(1, 2, 0)[hsel:hsel + 4].reshape(2, 128, L)
            .astype(np.float16))
        kt = np.ascontiguousarray(
            k4[b].transpose(1, 2, 0)[hsel:hsel + 4].reshape(2, 128, L)
            .astype(np.float16))
        vsl = v4[b, :, hsel:hsel + 4, :].transpose(1, 0, 2)  # [4, L, DH]
        # 64 ones columns: the PV matmul then writes the softmax denominator
        # to PSUM partitions 0..63 -- a free in-matmul broadcast (those
        # array columns would otherwise multiply zero padding), so the
        # normalize chain is just reciprocal + multiply on the DVE.
        # dh lives at partitions 64..127 (32-aligned AP bases; custom DVE
        # ops require base_partition 0 for the reciprocal input).
        vext = np.concatenate(
            [np.ones((PAIRS, L, 64), dtype=np.float32), vsl], axis=2)
        vext = (vext.reshape(PAIRS, L // KC, KC, 128)
                .transpose(0, 2, 1, 3).reshape(PAIRS, KC, -1))
        vext = np.ascontiguousarray(vext.astype(np.float16))
        wp = np.ascontiguousarray(
            w[(c % 4) * 256:(c % 4 + 1) * 256, :].reshape(2, 128, D)
            .astype(np.float16))
        in_maps.append({"qt": qt, "kt": kt, "v": vext, "masks": masks,
                        "w": wp})
    return in_maps


def kernel(query, key, value, w_proj, b_proj, n_head):
    from concourse.bass_utils import run_bass_kernel_spmd

    bias = np.asarray(b_proj, dtype=np.float32)
    in_maps = _prep_in_maps(query, key, value, w_proj)
    nc = _get_nc()
    res = run_bass_kernel_spmd(nc, in_maps, list(range(NCORES)))

    out = np.zeros((B, L, D), dtype=np.float32)
    for c in range(NCORES):
        out[c // 4] += res.results[c]["out"]
    out += bias[None, None, :]
    return out
